# revision 1
# baseline (speedup 1.0000x reference)
"""DynamicConv1dTBC Trainium2 Bass kernel.

Problem: x [T=2048, B=4, C=1024] f32, Wlin [240, 1024] f32.
  w = softmax(einsum('tbc,kc->tbk', x, Wlin).reshape(T,B,H=16,K=15), axis=-1)
  out[t,b,h,r] = sum_k w[t,b,h,k] * xpad[t+k, b, h*64+r]   (causal, PAD_L=14)

Sharding: T split across 8 cores (256 out-timesteps each + 14-row left halo).

Per-core algorithm (all timesteps local to the shard, x_ext has 270 rows):
  1. weight-gen: w_logits[t, j] = x[t] @ WlinT  (bf16 matmuls, lhsT = PE-transposed
     x chunks, rhs = PE-transposed Wlin), per (b, half-of-256-rows).
  2. softmax: exp on ACT; per-head sums via 3D reduce on DVE; reciprocal kept for
     a fused normalize at PSUM-evac time (conv is linear in w, so normalize last).
  3. unfold+contract as a banded matmul: out[t, r] = sum_s band[s, t]*x_ext[s+o, r]
     where band[s, t] = exp_w[o+t, s-t]. The shear (t,k)->(s,t) cannot be done as
     one strided DMA (contiguity), so: exp_w -> DRAM (cast bf16) -> single-partition
     zero-padded "wpad" (pitch 144) -> per-(b,h,chunk) pitch-trick DMA gives
     A_nat[t, s] -> PE transpose -> band[s, t] -> matmul with x view-2 slices.
  4. evac: PSUM conv result * (1/sum) via tensor_scalar, assembled and stored.
"""
import sys, os
for _p in ("/opt/trn_rl_repo",):
    if _p not in sys.path and os.path.isdir(_p):
        sys.path.insert(0, _p)

import numpy as np
from contextlib import ExitStack

import concourse.bass as bass
import concourse.tile as tile
from concourse import mybir, bacc, masks
from concourse._compat import with_exitstack
from concourse.bass_utils import run_bass_kernel_spmd

# ---- problem constants -------------------------------------------------------
T_GLOBAL, B, C = 2048, 4, 1024
H, K, R = 16, 15, 64
J = H * K                      # 240
PAD_L = K - 1                  # 14
N_CORES = 8
T_LOC = T_GLOBAL // N_CORES    # 256 output timesteps per core
T_EXT = T_LOC + PAD_L          # 270 input rows per core
OC = 2                         # out-chunks of 128 timesteps
TCH = T_LOC // OC              # 128
SW = TCH + PAD_L               # 142 s-window per chunk
PK = 144                       # wpad k-pitch (> TCH + PAD_L - 1 = 141)
F32 = mybir.dt.float32
BF16 = mybir.dt.bfloat16
VARIANT = "full"   # full | noband | noconv | xonly


@with_exitstack
def dynconv_prelude(ctx: ExitStack, tc: tile.TileContext, wpad_dram: dict):
    nc = tc.nc
    PLANE = TCH * PK
    zp = ctx.enter_context(tc.tile_pool(name="zfill", bufs=1))
    zsrc = zp.tile([TCH, H * (PK - K)], BF16)
    nc.gpsimd.memset(zsrc[:], 0.0)
    for (b, oc), wt in wpad_dram.items():
        wpd = wt[:]
        dst = bass.AP(wpd.tensor, wpd.offset + K,
                      [[PK, TCH], [PLANE, H], [1, PK - K]])
        nc.sync.dma_start(dst, zsrc[:])


def make_wpad_dram(pool):
    wpad_dram = {}
    for b in range(B):
        for oc in range(OC):
            wpad_dram[(b, oc)] = pool.tile([H, TCH * PK], BF16,
                                           name=f"wpd{b}_{oc}", tag=f"wpd{b}_{oc}")
    return wpad_dram


@with_exitstack
def dynconv_kernel(ctx: ExitStack, tc: tile.TileContext,
                   x_ap: bass.AP, wlin_ap: bass.AP, out_ap: bass.AP,
                   wpad_dram: dict = None):
    nc = tc.nc
    variant = VARIANT
    const = ctx.enter_context(tc.tile_pool(name="const", bufs=1))
    xpool = ctx.enter_context(tc.tile_pool(name="x", bufs=1))
    wl = ctx.enter_context(tc.tile_pool(name="wl", bufs=1))
    xtp = ctx.enter_context(tc.tile_pool(name="xt", bufs=3))
    wex = ctx.enter_context(tc.tile_pool(name="wex", bufs=2))
    wpadp = ctx.enter_context(tc.tile_pool(name="wpad", bufs=1))
    anat = ctx.enter_context(tc.tile_pool(name="anat", bufs=4))
    bandp = ctx.enter_context(tc.tile_pool(name="band", bufs=4))
    stg = ctx.enter_context(tc.tile_pool(name="stg", bufs=3))
    dram = ctx.enter_context(tc.tile_pool(name="dram", bufs=1, space="DRAM"))
    ps_w = ctx.enter_context(tc.tile_pool(name="psw", bufs=2, space="PSUM"))
    ps_t = ctx.enter_context(tc.tile_pool(name="pst", bufs=3, space="PSUM"))
    ps_c = ctx.enter_context(tc.tile_pool(name="psc", bufs=3, space="PSUM"))

    ident = const.tile([128, 128], BF16)
    masks.make_identity(nc, ident[:])

    # ---- WlinT build: Wlin [240,1024] -> per-cchunk [128 c, 240 j] bf16 ------
    wlin_b = wl.tile([120, C], BF16)
    nc.gpsimd.dma_start(wlin_b[:], wlin_ap[0:120, :])
    wlin_b2 = wl.tile([120, C], BF16)
    nc.gpsimd.dma_start(wlin_b2[:], wlin_ap[120:240, :])
    wlinT = []
    for cc in range(8):
        wt = wl.tile([128, J], BF16, name=f"wlinT{cc}", tag=f"wlinT{cc}")
        for i, wb in enumerate((wlin_b, wlin_b2)):
            pt = ps_t.tile([128, 120], BF16, tag="ptt")
            nc.tensor.transpose(pt[:], wb[:, cc * 128:(cc + 1) * 128],
                                ident[0:120, 0:120])
            nc.vector.tensor_copy(wt[:, i * 120:(i + 1) * 120], pt[:])
        wlinT.append(wt)

    # ---- x loads (cast f32 -> bf16 during DMA on gpsimd) ---------------------
    # view-2 layout [t, b*C]; tiles: Tpre = rows [0,14), T1 = [14,142), T2 = [142,270)
    xv = x_ap.rearrange("t b c -> t (b c)")
    x_pre = xpool.tile([PAD_L, B * C], BF16)
    x_mid = xpool.tile([PAD_L, B * C], BF16)
    x_t1b = [xpool.tile([TCH, C], BF16, name=f"xt1b{i}", tag=f"xt1b{i}")
             for i in range(B)]
    x_t2b = [xpool.tile([TCH, C], BF16, name=f"xt2b{i}", tag=f"xt2b{i}")
             for i in range(B)]
    for bb in range(B):
        cs = slice(bb * C, (bb + 1) * C)
        nc.gpsimd.dma_start(x_t1b[bb][:], xv[PAD_L:PAD_L + TCH, cs])
        nc.gpsimd.dma_start(x_t2b[bb][:], xv[PAD_L + TCH:T_EXT, cs])
        if bb == 0:
            nc.gpsimd.dma_start(x_pre[:], xv[0:PAD_L, :])
            nc.gpsimd.dma_start(x_mid[:], xv[TCH:TCH + PAD_L, :])
    x_tiles_b = [x_t1b, x_t2b]

    # ---- zero-padded staging tiles + per-(b,oc) DRAM wpads -------------------
    PLANE = TCH * PK
    wn_bufs = []
    for i in range(3):
        wz = wex.tile([TCH, H * K], BF16, name=f"wnz{i}", tag=f"wnz{i}")
        wn_bufs.append(wz)

    # ---- phase 1: weight-gen + softmax per (b, oc) ---------------------------
    for b in range(B):
        for oc in range(OC):
            if variant == "xonly":
                break
            xt_src = x_tiles_b[oc][b]
            # xT chunks: 8 PE-transposes into ONE psum bank, single evac
            pw = ps_w.tile([TCH, J], F32)
            ptx = ps_t.tile([128, 8 * TCH], BF16, tag="ptt")
            for cc in range(8):
                nc.tensor.matmul(ptx[:, cc * TCH:(cc + 1) * TCH],
                                 xt_src[:, cc * 128:(cc + 1) * 128], ident[:],
                                 is_transpose=True, skip_group_check=True)
            xTw = xtp.tile([128, 8 * TCH], BF16, tag="xTw")
            if (b + oc) % 3 == 2:
                nc.scalar.activation(xTw[:], ptx[:],
                                     mybir.ActivationFunctionType.Copy)
            else:
                nc.vector.tensor_copy(xTw[:], ptx[:])
            for cc in range(8):
                nc.tensor.matmul(pw[:], xTw[:, cc * TCH:(cc + 1) * TCH],
                                 wlinT[cc][:], start=(cc == 0), stop=(cc == 7))
            # softmax pieces
            we = wex.tile([TCH, J], F32)
            nc.scalar.activation(we[:], pw[:], mybir.ActivationFunctionType.Exp)
            sums = wex.tile([TCH, H], F32, tag="sums")
            nc.vector.reduce_sum(sums[:], we[:].rearrange("t (h k) -> t h k", k=K),
                                 axis=mybir.AxisListType.X)
            inv = wex.tile([TCH, H], F32, tag="invs")
            nc.vector.reciprocal(inv[:], sums[:])
            # normalize: w = wexp * inv (broadcast over k)
            wn = wn_bufs[(b * OC + oc) % 3]
            nc.vector.tensor_tensor(
                wn[:].rearrange("t (h k) -> t h k", k=K),
                we[:].rearrange("t (h k) -> t h k", k=K),
                inv[:].unsqueeze(2).broadcast_to((TCH, H, K)),
                op=mybir.AluOpType.mult)
            # stage only the k<15 columns (zeros pre-written once)
            wpd = wpad_dram[(b, oc)][:]
            dst = bass.AP(wpd.tensor, wpd.offset,
                          [[PK, TCH], [PLANE, H], [1, K]])
            src = bass.AP(wn[:].tensor, wn[:].offset,
                          [[H * K, TCH], [K, H], [1, K]])
            nc.sync.dma_start(dst, src)


    # ---- phase 3+4: bands + conv + fused evac per (b, h, oc) -----------------
    dummy_an = None
    if variant == "nobanddma":
        dummy_an = const.tile([TCH, H * SW], BF16, name="dummy_an")
        nc.gpsimd.memset(dummy_an[:], 0.001)
    if variant in ("noconv", "xonly"):
        for i in range(3):
            sz = stg.tile([TCH, C], F32, name=f"stz{i}", tag="stage")
            nc.gpsimd.memset(sz[:], 0.0)
    dummy_bb = None
    if variant in ("noband", "noconv"):
        dummy_bb = const.tile([128, TCH], BF16, name="dummy_bb")
        nc.gpsimd.memset(dummy_bb[:], 0.001)
        dummy_ba = const.tile([PAD_L, TCH], BF16, name="dummy_ba")
        nc.gpsimd.memset(dummy_ba[:], 0.001)
    for b in range(B):
        for oc in range(OC):
            stage = stg.tile([TCH, C], F32)
            if variant in ("noconv", "xonly"):
                nc.sync.dma_start(out_ap[oc * TCH:(oc + 1) * TCH, b, :], stage[:])
                continue
            if variant == "nobanddma":
                anm = dummy_an
            elif variant != "noband":
                anm = anat.tile([TCH, H * SW], BF16, tag="anm")
                wpd = wpad_dram[(b, oc)][:]
                srcm = bass.AP(wpd.tensor, wpd.offset,
                               [[PK - 1, TCH], [PLANE, H], [1, SW]])
                nc.sync.dma_start(anm[:], srcm)
            pc_banks = [ps_c.tile([TCH, 8 * R], F32, name=f"pcb{q}", tag="pcb")
                        for q in range(2)]
            bw_tiles = {}
            for hq in range(2):
                if variant in ("noband",):
                    break
                pbq = ps_t.tile([128, 8 * TCH], BF16, tag="ptt", name=f"pbq{hq}")
                for j in range(8):
                    h = hq * 8 + j
                    nc.tensor.matmul(
                        pbq[:, j * TCH:(j + 1) * TCH],
                        anm[:, h * SW + PAD_L:h * SW + SW], ident[:],
                        is_transpose=True, skip_group_check=True)
                bwq = bandp.tile([128, 8 * TCH], BF16, tag="bw", name=f"bw{hq}")
                if hq == 0:
                    nc.vector.tensor_copy(bwq[:], pbq[:])
                else:
                    nc.scalar.activation(bwq[:], pbq[:],
                                         mybir.ActivationFunctionType.Copy)
                bw_tiles[hq] = bwq
            for h in range(H):
                p = b * H + h
                o0 = oc * TCH
                if variant == "noband":
                    bb, ba = dummy_bb, dummy_ba
                    pc = ps_c.tile([TCH, R], F32)
                    fo = b * C + h * R
                    if oc == 0:
                        nc.tensor.matmul(pc[:], ba[:], x_pre[:, fo:fo + R],
                                         start=True, stop=False)
                        nc.tensor.matmul(pc[:], bb[:], x_t1b[b][:, h * R:h * R + R],
                                         start=False, stop=True)
                    else:
                        nc.tensor.matmul(pc[:], ba[:], x_mid[:, fo:fo + R],
                                         start=True, stop=False)
                        nc.tensor.matmul(pc[:], bb[:], x_t2b[b][:, h * R:h * R + R],
                                         start=False, stop=True)
                    if h % 3 != 2:
                        nc.scalar.activation(stage[:, h * R:(h + 1) * R], pc[:],
                                             mybir.ActivationFunctionType.Copy)
                    else:
                        nc.vector.tensor_copy(stage[:, h * R:(h + 1) * R], pc[:])
                    continue
                bb = bw_tiles[h // 8][:, (h % 8) * TCH:(h % 8 + 1) * TCH]
                ba = bandp.tile([PAD_L, TCH], BF16, tag="ba")
                pa = ps_t.tile([PAD_L, TCH], BF16, tag="ptt")
                nc.tensor.transpose(pa[:], anm[:, h * SW:h * SW + PAD_L],
                                    ident[:])
                nc.vector.tensor_copy(ba[:], pa[:])
                # conv matmuls: out[t, r] = sum_s band[s, t] * x_ext[o0+s, b, h*64+r]
                pc = pc_banks[h // 8][:, (h % 8) * R:(h % 8 + 1) * R]
                fo = b * C + h * R
                fl = h * R
                if oc == 0:
                    nc.tensor.matmul(pc, ba[:], x_pre[:, fo:fo + R],
                                     start=True, stop=False,
                                     skip_group_check=True)
                    nc.tensor.matmul(pc, bb, x_t1b[b][:, fl:fl + R],
                                     start=False, stop=True,
                                     skip_group_check=True)
                else:
                    nc.tensor.matmul(pc, ba[:], x_mid[:, fo:fo + R],
                                     start=True, stop=False,
                                     skip_group_check=True)
                    nc.tensor.matmul(pc, bb, x_t2b[b][:, fl:fl + R],
                                     start=False, stop=True,
                                     skip_group_check=True)
                if h % 8 == 7:
                    q = h // 8
                    if q == 0:
                        nc.vector.tensor_copy(
                            stage[:, q * 8 * R:(q + 1) * 8 * R], pc_banks[q][:])
                    else:
                        nc.scalar.activation(
                            stage[:, q * 8 * R:(q + 1) * 8 * R], pc_banks[q][:],
                            mybir.ActivationFunctionType.Copy)
            # store stage -> out[oc*128:(oc+1)*128, b, :]
            nc.sync.dma_start(out_ap[oc * TCH:(oc + 1) * TCH, b, :], stage[:])


def build_program(debug=False, reps=1):
    nc = bacc.Bacc("TRN2", target_bir_lowering=False, debug=debug,
                   enable_asserts=False, num_devices=N_CORES)
    x_t = nc.dram_tensor("x", [T_EXT, B, C], F32, kind="ExternalInput")
    wlin_t = nc.dram_tensor("wlin", [J, C], F32, kind="ExternalInput")
    out_t = nc.dram_tensor("out", [T_LOC, B, C], F32, kind="ExternalOutput")
    with tile.TileContext(nc) as tc:
        with tc.tile_pool(name="wpddram", bufs=1, space="DRAM") as wpool:
            wpad_dram = make_wpad_dram(wpool)
            if os.environ.get("NOPRELUDE") != "1":
                dynconv_prelude(tc, wpad_dram)
            if reps == 1:
                dynconv_kernel(tc, x_t.ap(), wlin_t.ap(), out_t.ap(),
                               wpad_dram=wpad_dram)
            else:
                with tc.For_i(0, reps, 1):
                    dynconv_kernel(tc, x_t.ap(), wlin_t.ap(), out_t.ap(),
                                   wpad_dram=wpad_dram)
    nc.compile()
    return nc


_NC_CACHE = None


def kernel(x: np.ndarray, Wlin: np.ndarray) -> np.ndarray:
    global _NC_CACHE
    if _NC_CACHE is None:
        _NC_CACHE = build_program()
    nc = _NC_CACHE
    xp = np.pad(x, ((PAD_L, 0), (0, 0), (0, 0)))
    in_maps = []
    for i in range(N_CORES):
        in_maps.append({
            "x": np.ascontiguousarray(xp[i * T_LOC:i * T_LOC + T_EXT]),
            "wlin": np.ascontiguousarray(Wlin),
        })
    res = run_bass_kernel_spmd(nc, in_maps, core_ids=list(range(N_CORES)))
    outs = [res.results[i]["out"] for i in range(N_CORES)]
    return np.concatenate(outs, axis=0)

